# revision 37
# baseline (speedup 1.0000x reference)
"""Trainium2 Bass kernel for nn_DynamicsLookAheadModel.

LSTM warm-up over S=96 steps + 32-step look-ahead with output feedback,
data-parallel over the batch (2048) across 8 NeuronCores (256 per core).

Per-core layout (all fp32):
  - Everything "transposed": hidden units on partitions, batch on the free dim.
    H=256 tensors are folded into [128, 512] tiles:
      phys[p, j]       = logical[p,     j]   for j <  256   (h-dims 0..127)
      phys[p, 256 + j] = logical[128+p, j]                  (h-dims 128..255)
  - Gates g = W_ih@x + W_hh@h computed on the PE into PSUM; the K=32
    x-projection runs as 4 concurrent quadrant matmuls (tile_position row
    packing, x and W_ih replicated across the 4 row bands), the K=256 h part
    as 2 full K=128 accumulation matmuls per M-tile.
  - PSUM: gate M-tile m (of 8) lives in bank m%4, columns 256*(m//4).
  - Bias (b_ih+b_hh) applied via the ScalarE activation bias port (per
    partition), one Sigmoid/Tanh instruction per M-tile from PSUM.
  - Cell update on VectorE in fp32.
  - STE binarization uses sign(c') (sigmoid(o) > 0, tanh sign-preserving):
    bits = (c' > 0) via one tensor_scalar is_gt.
  - Outputs are stored per step as [6, 256], assembled as [33, 6, 256] in
    DRAM; the host gather transposes to [256, 33, 6].
"""

import os

import numpy as np

import concourse.bass as bass
import concourse.mybir as mybir
import concourse.tile as tile
from concourse.bass_utils import run_bass_kernel_spmd

B, S, F, H, O = 2048, 96, 32, 256, 6
LA = 32
NCORES = 8
BL = B // NCORES  # 256 per-core batch
FD = 2 * BL  # 512: folded free dim for H=256 tensors
FP32 = mybir.dt.float32
# Matmul operand dtype: fp32 required — 16-bit quantization noise gets
# amplified by the STE binarization to several percent output error (measured).
MM16 = os.environ.get("MM16", "0") == "1"
MMDT = mybir.dt.float16 if MM16 else FP32
MMNP = np.float16 if MM16 else np.float32


# --- workaround: this walrus build allows only ONE sem wait per instruction ---
# Spill excess semaphore waits onto same-engine NOPs placed just before the
# instruction (engines execute their queue in order, so semantics match).
def _spill_excess_waits(nc, limit=1):
    cnt = 0
    for f in nc.m.functions:
        for bb in f.blocks:
            new_list = []
            for ins in bb.instructions:
                si = ins.sync_info
                if si and si.on_wait and len(si.on_wait) > limit:
                    waits = list(si.on_wait)
                    for w in waits[:-limit]:
                        n = mybir.InstNoOp(name=f"wspill_{cnt}", ins=[], outs=[])
                        cnt += 1
                        n.engine = ins.engine
                        n.sync_info = mybir.SyncInfo(on_wait=[w], on_update=[])
                        new_list.append(n)
                    ins.sync_info = mybir.SyncInfo(
                        on_wait=waits[-limit:], on_update=list(si.on_update)
                    )
                new_list.append(ins)
            bb.instructions[:] = new_list
    return cnt


def build_nc(n_warm=S, n_la=LA, spill=True):
    from contextlib import ExitStack

    nc = bass.Bass()
    AF = mybir.ActivationFunctionType
    ALU = mybir.AluOpType

    # warmup x: transposed, step-PAIRED on the free dim (N=512 per pair),
    # bf16 hi/lo split, replicated over the 4 PE row bands.
    assert n_warm % 2 == 0
    xrh_d = nc.dram_tensor(
        "xrh", [n_warm // 2, 128, 2 * BL], mybir.dt.bfloat16, kind="ExternalInput"
    )
    xrl_d = nc.dram_tensor(
        "xrl", [n_warm // 2, 128, 2 * BL], mybir.dt.bfloat16, kind="ExternalInput"
    )
    # LA x: fp32, unpaired (rows 0:6 get the fed-back output)
    xla_d = nc.dram_tensor("xla", [n_la, 128, BL], FP32, kind="ExternalInput")
    # W_ih.T [32, 1024] bf16 hi/lo + fp32, each replicated over the 4 bands
    wrh_d = nc.dram_tensor("wrh", [128, 4 * H], mybir.dt.bfloat16, kind="ExternalInput")
    wrl_d = nc.dram_tensor("wrl", [128, 4 * H], mybir.dt.bfloat16, kind="ExternalInput")
    wrep_d = nc.dram_tensor("wrep", [128, 4 * H], FP32, kind="ExternalInput")
    BF16 = mybir.dt.bfloat16
    # W_hh in split bf16 (hi + residual lo): bf16 products are exact in the
    # fp32 PSUM accumulation, so Whi@hhi + Whi@hlo + Wlo@hhi reproduces the
    # fp32 matmul to ~1e-6 while running 1 cycle/row instead of fp32's 4.
    whh0h_d = nc.dram_tensor("whh0h", [128, 4 * H], BF16, kind="ExternalInput")
    whh0l_d = nc.dram_tensor("whh0l", [128, 4 * H], BF16, kind="ExternalInput")
    whh1h_d = nc.dram_tensor("whh1h", [128, 4 * H], BF16, kind="ExternalInput")
    whh1l_d = nc.dram_tensor("whh1l", [128, 4 * H], BF16, kind="ExternalInput")
    wfch_d = nc.dram_tensor("wfch", [128, 2 * O], mybir.dt.bfloat16, kind="ExternalInput")
    wfcl_d = nc.dram_tensor("wfcl", [128, 2 * O], mybir.dt.bfloat16, kind="ExternalInput")
    bias8_d = nc.dram_tensor("bias8", [128, 8], FP32, kind="ExternalInput")
    bfc_d = nc.dram_tensor("bfc", [O, 1], FP32, kind="ExternalInput")
    out_d = nc.dram_tensor("out_t", [n_la + 1, O, BL], FP32, kind="ExternalOutput")

    with tile.TileContext(nc) as tc, ExitStack() as es:
        wp_ctx = es.enter_context(tc.tile_pool(name="weights", bufs=1))
        xp_ctx = es.enter_context(tc.tile_pool(name="xtiles", bufs=1))
        sp_ctx = es.enter_context(tc.tile_pool(name="state", bufs=2))
        gp_ctx = es.enter_context(tc.tile_pool(name="gates", bufs=1, space="PSUM"))

        wrep = wp_ctx.tile([128, 4 * H], FP32, tag="wrep")
        nc.sync.dma_start(out=wrep, in_=wrep_d[:, :])
        wrh = wp_ctx.tile([128, 4 * H], BF16, tag="wrh")
        nc.sync.dma_start(out=wrh, in_=wrh_d[:, :])
        wrl = wp_ctx.tile([128, 4 * H], BF16, tag="wrl")
        nc.sync.dma_start(out=wrl, in_=wrl_d[:, :])
        whh0h = wp_ctx.tile([128, 4 * H], BF16, tag="whh0h")
        nc.sync.dma_start(out=whh0h, in_=whh0h_d[:, :])
        whh0l = wp_ctx.tile([128, 4 * H], BF16, tag="whh0l")
        nc.sync.dma_start(out=whh0l, in_=whh0l_d[:, :])
        whh1h = wp_ctx.tile([128, 4 * H], BF16, tag="whh1h")
        nc.sync.dma_start(out=whh1h, in_=whh1h_d[:, :])
        whh1l = wp_ctx.tile([128, 4 * H], BF16, tag="whh1l")
        nc.sync.dma_start(out=whh1l, in_=whh1l_d[:, :])
        whh = [(whh0h, whh0l), (whh1h, whh1l)]
        wfch = wp_ctx.tile([128, 2 * O], BF16, tag="wfch")
        nc.sync.dma_start(out=wfch, in_=wfch_d[:, :])
        wfcl = wp_ctx.tile([128, 2 * O], BF16, tag="wfcl")
        nc.sync.dma_start(out=wfcl, in_=wfcl_d[:, :])
        bias8 = wp_ctx.tile([128, 8], FP32, tag="bias8")
        nc.sync.dma_start(out=bias8, in_=bias8_d[:, :])
        bfc = wp_ctx.tile([O, 1], FP32, tag="bfc")
        nc.sync.dma_start(out=bfc, in_=bfc_d[:, :])

        # warmup x pair tiles (bf16 hi/lo)
        xpt = []
        for p in range(n_warm // 2):
            xth = xp_ctx.tile([128, 2 * BL], BF16, tag=f"xh{p}")
            nc.sync.dma_start(out=xth, in_=xrh_d[p, :, :])
            xtl = xp_ctx.tile([128, 2 * BL], BF16, tag=f"xl{p}")
            nc.sync.dma_start(out=xtl, in_=xrl_d[p, :, :])
            xpt.append((xth, xtl))
        # LA x tiles (fp32; rows 0:6 of each band get the fed-back output)
        xlat = []
        for k in range(n_la):
            xtile = xp_ctx.tile([128, BL], FP32, tag=f"xla{k}")
            nc.sync.dma_start(out=xtile, in_=xla_d[k, :, :])
            xlat.append(xtile)

        c_prev = None  # step 0 skips the f gate entirely
        h_prev = None  # step 0 skips the W_hh matmuls entirely

        # gate order i,f,g,o over M-tiles m=0..7 (gate X -> tiles 2X, 2X+1)
        # PSUM: one bank per M-tile (separate accumulation state per tile)
        GATE_FUNC = [AF.Sigmoid, AF.Sigmoid, AF.Tanh, AF.Sigmoid]  # i, f, g, o
        EMIT_ORDER = [2, 1, 0, 3]  # g, f, i, o

        def alloc_banks():
            banks = []
            for b in range(8):
                pbank = gp_ctx.tile([128, FD], FP32, tag=f"pb{b}")
                banks.append(pbank)
            return banks

        def gates_pair(xpair):
            # x-projection for TWO steps at once (N=512), quadrant-packed,
            # as three exact bf16 hi/lo products
            xth, xtl = xpair
            banks = alloc_banks()
            prods = [(wrh, xth), (wrl, xth), (wrh, xtl)]
            for j, (w_t, x_t) in enumerate(prods):
                for grp in (0, 1):
                    for band in range(4):
                        m = 4 * grp + band
                        nc.tensor.matmul(
                            banks[m][:, :],
                            w_t[32 * band : 32 * band + 32, 128 * m : 128 * m + 128],
                            x_t[32 * band : 32 * band + 32, :],
                            start=(j == 0),
                            stop=False,
                            tile_position=(32 * band, 0),
                            skip_group_check=True,
                        )
            return banks

        def x_mms(banks, xslice, first=False, x_last=False):
            # x_last: the x matmuls close accumulation groups the h matmuls
            # opened (LA phase: the o-dependent x wait hides under the h part)
            for grp in (0, 1):
                for band in range(4):
                    m = 4 * grp + band
                    if first and m in (2, 3):
                        continue  # f gate unused at step 0
                    nc.tensor.matmul(
                        banks[m][:, 0:BL],
                        wrep[32 * band : 32 * band + 32, 128 * m : 128 * m + 128],
                        xslice[32 * band : 32 * band + 32, :],
                        start=not x_last,
                        stop=first or x_last,
                        tile_position=(32 * band, 0),
                        skip_group_check=True,
                    )

        def gates_single(xpair, off, first=False):
            # warmup unpaired step: bf16 hi/lo products on a pair half
            xth, xtl = xpair
            banks = alloc_banks()
            prods = [(wrh, xth), (wrl, xth), (wrh, xtl)]
            for j, (w_t, x_t) in enumerate(prods):
                for grp in (0, 1):
                    for band in range(4):
                        m = 4 * grp + band
                        if first and m in (2, 3):
                            continue  # f gate unused at step 0
                        nc.tensor.matmul(
                            banks[m][:, 0:BL],
                            w_t[32 * band : 32 * band + 32, 128 * m : 128 * m + 128],
                            x_t[32 * band : 32 * band + 32, off : off + BL],
                            start=(j == 0),
                            stop=(first and j == 2),
                            tile_position=(32 * band, 0),
                            skip_group_check=True,
                        )
            return banks

        def gates_la(xtile):
            banks = alloc_banks()
            x_mms(banks, xtile)
            return banks

        def lstm_tail(banks, off, h_prev, c_prev, first=False, h_starts=False,
                      x_emit=None, want_bits=False):
            def psl(m):
                return banks[m][:, off : off + BL]

            # h part: per emission-ordered gate, 2 M-tiles x 2 K-tiles.
            # h lives in two half tiles so k0 matmuls start as soon as the
            # low half of the tail finishes. With h_starts=True the k0 matmul
            # opens the accumulation group (LA: x part accumulates last).
            if not first:
                for g in EMIT_ORDER:
                    for m in (2 * g, 2 * g + 1):
                        col = 128 * m
                        for k in (0, 1):
                            wh, wl = whh[k]
                            hhi, hlo = h_prev[k]
                            prods = [(wh, hhi), (wl, hhi), (wh, hlo)]
                            for j, (w_t, h_t) in enumerate(prods):
                                last = k == 1 and j == 2
                                nc.tensor.matmul(
                                    psl(m),
                                    w_t[:, col : col + 128],
                                    h_t[:, :],
                                    start=(h_starts and k == 0 and j == 0),
                                    stop=(last and not h_starts),
                                    skip_group_check=True,
                                )
            if x_emit is not None:
                x_emit()

            # activations: one instr per M-tile into per-half tiles, bias via
            # the ACT bias port
            act = {}
            for g in EMIT_ORDER:
                if first and g == 1:
                    continue
                for half in (0, 1):
                    m = 2 * g + half
                    ah = sp_ctx.tile([128, BL], FP32, tag=f"a{g}_{half}")
                    act[(g, half)] = ah
                    nc.scalar.activation(
                        out=ah,
                        in_=psl(m),
                        func=GATE_FUNC[g],
                        bias=bias8[:, m : m + 1],
                    )

            # elementwise tail, low half first so h_lo lands early
            c_new = []
            h_new = []
            bits_new = []
            for half in (0, 1):
                cn = sp_ctx.tile([128, BL], FP32, tag=f"c{half}")
                if first:
                    nc.vector.tensor_tensor(
                        out=cn, in0=act[(0, half)], in1=act[(2, half)], op=ALU.mult
                    )
                else:
                    t1 = sp_ctx.tile([128, BL], FP32, tag=f"t1_{half}")
                    nc.vector.tensor_tensor(
                        out=t1, in0=act[(1, half)], in1=c_prev[half], op=ALU.mult
                    )
                    t2 = sp_ctx.tile([128, BL], FP32, tag=f"t2_{half}")
                    nc.vector.tensor_tensor(
                        out=t2, in0=act[(0, half)], in1=act[(2, half)], op=ALU.mult
                    )
                    nc.vector.tensor_tensor(out=cn, in0=t1, in1=t2, op=ALU.add)
                c_new.append(cn)
                if want_bits:
                    bt = sp_ctx.tile([128, BL], BF16, tag=f"bits{half}")
                    nc.vector.tensor_scalar(
                        out=bt, in0=cn, scalar1=0.0, scalar2=None, op0=ALU.is_gt
                    )
                    bits_new.append(bt)
                tc_h = sp_ctx.tile([128, BL], FP32, tag=f"tc{half}")
                nc.scalar.activation(out=tc_h, in_=cn, func=AF.Tanh)
                hhi = sp_ctx.tile([128, BL], BF16, tag=f"hhi{half}")
                nc.vector.tensor_tensor(
                    out=hhi, in0=act[(3, half)], in1=tc_h, op=ALU.mult
                )
                hn = sp_ctx.tile([128, BL], FP32, tag=f"h{half}")
                nc.vector.tensor_tensor(out=hn, in0=act[(3, half)], in1=tc_h, op=ALU.mult)
                hlo = sp_ctx.tile([128, BL], BF16, tag=f"hlo{half}")
                nc.vector.scalar_tensor_tensor(
                    out=hlo,
                    in0=hhi,
                    scalar=-1.0,
                    in1=hn,
                    op0=ALU.mult,
                    op1=ALU.add,
                )
                h_new.append((hhi, hlo))
            return h_new, c_new, bits_new

        def emit_output(k, bits_cur):
            # bits = (c' > 0); equals STE(h) since sigmoid(o)>0, tanh sign-pres.
            # po reuses a gate bank slot: all gate reads of this step are
            # done before bits is ready, so the WAR dep is satisfied
            po = gp_ctx.tile([O, BL], FP32, tag="pb0")
            for half in (0, 1):
                for j, w_t in enumerate((wfch, wfcl)):
                    nc.tensor.matmul(
                        po,
                        w_t[:, O * half : O * half + O],
                        bits_cur[half][:, :],
                        start=(half == 0 and j == 0),
                        stop=(half == 1 and j == 1),
                        skip_group_check=True,
                    )
            osb = sp_ctx.tile([O, BL], FP32, tag="osb")
            nc.scalar.activation(out=osb, in_=po, func=AF.Identity, bias=bfc)
            nc.sync.dma_start(out=out_d[k, :, :], in_=osb)
            return osb

        # steps 0 and 1 unpaired (step 0 has no h part), pairs from step 2
        bk = gates_single(xpt[0], 0, first=True)
        h_prev, c_prev, _ = lstm_tail(bk, 0, None, None, first=True)
        bk = gates_single(xpt[0], BL)
        h_prev, c_prev, _ = lstm_tail(bk, 0, h_prev, c_prev)
        for p in range(1, n_warm // 2):
            bk = gates_pair(xpt[p])
            h_prev, c_prev, _ = lstm_tail(bk, 0, h_prev, c_prev)
            h_prev, c_prev, bits = lstm_tail(
                bk, BL, h_prev, c_prev, want_bits=(p == n_warm // 2 - 1)
            )

        for k in range(n_la + 1):
            osb = emit_output(k, bits)
            if k < n_la:
                xv = xlat[k]
                for band in range(4):
                    dst = xv[32 * band : 32 * band + O, :]
                    if band % 2 == 0:
                        nc.vector.tensor_copy(out=dst, in_=osb)
                    else:
                        nc.scalar.copy(out=dst, in_=osb)
                bk = gates_la(xv)
                h_prev, c_prev, bits = lstm_tail(
                    bk, 0, h_prev, c_prev, want_bits=True
                )

    if spill:
        _spill_excess_waits(nc)
    return nc


def _host_prep(x, W_ih, W_hh, b_ih, b_hh, W_fc, b_fc):
    """Build the 8 per-core input maps."""
    x = np.asarray(x, dtype=np.float32)
    W_ih = np.asarray(W_ih, dtype=np.float32)
    W_hh = np.asarray(W_hh, dtype=np.float32)
    b_ih = np.asarray(b_ih, dtype=np.float32)
    b_hh = np.asarray(b_hh, dtype=np.float32)
    W_fc = np.asarray(W_fc, dtype=np.float32)
    b_fc = np.asarray(b_fc, dtype=np.float32)
    import ml_dtypes as mld

    bias = (b_ih + b_hh).astype(np.float32)
    w32t = np.ascontiguousarray(W_ih.T).astype(np.float32)  # [32, 1024]
    wrep = np.ascontiguousarray(np.tile(w32t, (4, 1)))  # [128, 1024]
    w_hi = w32t.astype(mld.bfloat16)
    w_lo = (w32t - w_hi.astype(np.float32)).astype(mld.bfloat16)
    import ml_dtypes

    whh_t = np.ascontiguousarray(W_hh.T).astype(np.float32)  # [256, 1024]
    whh_hi = whh_t.astype(ml_dtypes.bfloat16)
    whh_lo = (whh_t - whh_hi.astype(np.float32)).astype(ml_dtypes.bfloat16)
    wfc_fold = np.concatenate([W_fc.T[:128], W_fc.T[128:]], axis=1).astype(np.float32)
    wfc_hi = wfc_fold.astype(mld.bfloat16)
    wfc_lo = (wfc_fold - wfc_hi.astype(np.float32)).astype(mld.bfloat16)
    shared = {
        "wrep": wrep,
        "wrh": np.ascontiguousarray(np.tile(w_hi, (4, 1))),
        "wrl": np.ascontiguousarray(np.tile(w_lo, (4, 1))),
        "whh0h": np.ascontiguousarray(whh_hi[:128]),
        "whh0l": np.ascontiguousarray(whh_lo[:128]),
        "whh1h": np.ascontiguousarray(whh_hi[128:]),
        "whh1l": np.ascontiguousarray(whh_lo[128:]),
        "wfch": np.ascontiguousarray(wfc_hi),
        "wfcl": np.ascontiguousarray(wfc_lo),
        "bias8": np.ascontiguousarray(bias.reshape(8, 128).T).astype(np.float32),
        "bfc": np.ascontiguousarray(b_fc.reshape(O, 1)).astype(np.float32),
    }
    in_maps = []
    for c in range(NCORES):
        xc = x[c * BL : (c + 1) * BL]  # [BL, S, F]
        xT = np.ascontiguousarray(xc.transpose(1, 2, 0)).astype(np.float32)
        xpair = (
            xT.reshape(S // 2, 2, F, BL).transpose(0, 2, 1, 3).reshape(S // 2, F, 2 * BL)
        )
        x_hi = xpair.astype(mld.bfloat16)
        x_lo = (xpair - x_hi.astype(np.float32)).astype(mld.bfloat16)
        xla = np.ascontiguousarray(np.tile(xT[:LA], (1, 4, 1)))  # [LA, 128, BL]
        in_maps.append(
            {
                "xrh": np.ascontiguousarray(np.tile(x_hi, (1, 4, 1))),
                "xrl": np.ascontiguousarray(np.tile(x_lo, (1, 4, 1))),
                "xla": xla,
                **shared,
            }
        )
    return in_maps


_NC_CACHE = {}


def _get_nc():
    if "nc" not in _NC_CACHE:
        _NC_CACHE["nc"] = build_nc()
    return _NC_CACHE["nc"]


def run(inputs, trace=False):
    in_maps = _host_prep(**inputs)
    nc = _get_nc()
    res = run_bass_kernel_spmd(nc, in_maps, core_ids=list(range(NCORES)), trace=trace)
    outs = []
    for c in range(NCORES):
        o = res.results[c]["out_t"]  # [33, 6, BL]
        outs.append(np.ascontiguousarray(o.transpose(2, 0, 1)))  # [BL, 33, 6]
    full = np.concatenate(outs, axis=0).astype(np.float32)  # [B, 33, 6]
    return full, res


def kernel(**inputs):
    full, _ = run(inputs, trace=False)
    return full


if __name__ == "__main__":
    t = build_nc()
    print("built ok")


# revision 38
# speedup vs baseline: 1.0250x; 1.0250x over previous
"""Trainium2 Bass kernel for nn_DynamicsLookAheadModel.

LSTM warm-up over S=96 steps + 32-step look-ahead with output feedback,
data-parallel over the batch (2048) across 8 NeuronCores (256 per core).

Per-core layout (all fp32):
  - Everything "transposed": hidden units on partitions, batch on the free dim.
    H=256 tensors are folded into [128, 512] tiles:
      phys[p, j]       = logical[p,     j]   for j <  256   (h-dims 0..127)
      phys[p, 256 + j] = logical[128+p, j]                  (h-dims 128..255)
  - Gates g = W_ih@x + W_hh@h computed on the PE into PSUM; the K=32
    x-projection runs as 4 concurrent quadrant matmuls (tile_position row
    packing, x and W_ih replicated across the 4 row bands), the K=256 h part
    as 2 full K=128 accumulation matmuls per M-tile.
  - PSUM: gate M-tile m (of 8) lives in bank m%4, columns 256*(m//4).
  - Bias (b_ih+b_hh) applied via the ScalarE activation bias port (per
    partition), one Sigmoid/Tanh instruction per M-tile from PSUM.
  - Cell update on VectorE in fp32.
  - STE binarization uses sign(c') (sigmoid(o) > 0, tanh sign-preserving):
    bits = (c' > 0) via one tensor_scalar is_gt.
  - Outputs are stored per step as [6, 256], assembled as [33, 6, 256] in
    DRAM; the host gather transposes to [256, 33, 6].
"""

import os

import numpy as np

import concourse.bass as bass
import concourse.mybir as mybir
import concourse.tile as tile
from concourse.bass_utils import run_bass_kernel_spmd

B, S, F, H, O = 2048, 96, 32, 256, 6
LA = 32
NCORES = 8
BL = B // NCORES  # 256 per-core batch
FD = 2 * BL  # 512: folded free dim for H=256 tensors
FP32 = mybir.dt.float32
# Matmul operand dtype: fp32 required — 16-bit quantization noise gets
# amplified by the STE binarization to several percent output error (measured).
MM16 = os.environ.get("MM16", "0") == "1"
MMDT = mybir.dt.float16 if MM16 else FP32
MMNP = np.float16 if MM16 else np.float32


# --- workaround: this walrus build allows only ONE sem wait per instruction ---
# Spill excess semaphore waits onto same-engine NOPs placed just before the
# instruction (engines execute their queue in order, so semantics match).
def _spill_excess_waits(nc, limit=1):
    cnt = 0
    for f in nc.m.functions:
        for bb in f.blocks:
            new_list = []
            for ins in bb.instructions:
                si = ins.sync_info
                if si and si.on_wait and len(si.on_wait) > limit:
                    waits = list(si.on_wait)
                    for w in waits[:-limit]:
                        n = mybir.InstNoOp(name=f"wspill_{cnt}", ins=[], outs=[])
                        cnt += 1
                        n.engine = ins.engine
                        n.sync_info = mybir.SyncInfo(on_wait=[w], on_update=[])
                        new_list.append(n)
                    ins.sync_info = mybir.SyncInfo(
                        on_wait=waits[-limit:], on_update=list(si.on_update)
                    )
                new_list.append(ins)
            bb.instructions[:] = new_list
    return cnt


def build_nc(n_warm=S, n_la=LA, spill=True):
    from contextlib import ExitStack

    nc = bass.Bass()
    AF = mybir.ActivationFunctionType
    ALU = mybir.AluOpType

    # warmup x: transposed, step-PAIRED on the free dim (N=512 per pair),
    # bf16 hi/lo split, replicated over the 4 PE row bands.
    assert n_warm % 2 == 0
    xrh_d = nc.dram_tensor(
        "xrh", [n_warm // 2, 128, 2 * BL], mybir.dt.bfloat16, kind="ExternalInput"
    )
    xrl_d = nc.dram_tensor(
        "xrl", [n_warm // 2, 128, 2 * BL], mybir.dt.bfloat16, kind="ExternalInput"
    )
    # LA x: fp32, unpaired (rows 0:6 get the fed-back output)
    xla_d = nc.dram_tensor("xla", [n_la, 128, BL], FP32, kind="ExternalInput")
    # W_ih.T [32, 1024] bf16 hi/lo + fp32, each replicated over the 4 bands
    wrh_d = nc.dram_tensor("wrh", [128, 4 * H], mybir.dt.bfloat16, kind="ExternalInput")
    wrl_d = nc.dram_tensor("wrl", [128, 4 * H], mybir.dt.bfloat16, kind="ExternalInput")
    wrep_d = nc.dram_tensor("wrep", [128, 4 * H], FP32, kind="ExternalInput")
    BF16 = mybir.dt.bfloat16
    # W_hh in split bf16 (hi + residual lo): bf16 products are exact in the
    # fp32 PSUM accumulation, so Whi@hhi + Whi@hlo + Wlo@hhi reproduces the
    # fp32 matmul to ~1e-6 while running 1 cycle/row instead of fp32's 4.
    whh0h_d = nc.dram_tensor("whh0h", [128, 4 * H], BF16, kind="ExternalInput")
    whh0l_d = nc.dram_tensor("whh0l", [128, 4 * H], BF16, kind="ExternalInput")
    whh1h_d = nc.dram_tensor("whh1h", [128, 4 * H], BF16, kind="ExternalInput")
    whh1l_d = nc.dram_tensor("whh1l", [128, 4 * H], BF16, kind="ExternalInput")
    wfc_d = nc.dram_tensor("wfc", [128, 2 * O], MMDT, kind="ExternalInput")
    bias8_d = nc.dram_tensor("bias8", [128, 8], FP32, kind="ExternalInput")
    bfc_d = nc.dram_tensor("bfc", [O, 1], FP32, kind="ExternalInput")
    out_d = nc.dram_tensor("out_t", [n_la + 1, O, BL], FP32, kind="ExternalOutput")

    with tile.TileContext(nc) as tc, ExitStack() as es:
        wp_ctx = es.enter_context(tc.tile_pool(name="weights", bufs=1))
        xp_ctx = es.enter_context(tc.tile_pool(name="xtiles", bufs=1))
        sp_ctx = es.enter_context(tc.tile_pool(name="state", bufs=2))
        gp_ctx = es.enter_context(tc.tile_pool(name="gates", bufs=1, space="PSUM"))

        wrep = wp_ctx.tile([128, 4 * H], FP32, tag="wrep")
        nc.sync.dma_start(out=wrep, in_=wrep_d[:, :])
        wrh = wp_ctx.tile([128, 4 * H], BF16, tag="wrh")
        nc.sync.dma_start(out=wrh, in_=wrh_d[:, :])
        wrl = wp_ctx.tile([128, 4 * H], BF16, tag="wrl")
        nc.sync.dma_start(out=wrl, in_=wrl_d[:, :])
        whh0h = wp_ctx.tile([128, 4 * H], BF16, tag="whh0h")
        nc.sync.dma_start(out=whh0h, in_=whh0h_d[:, :])
        whh0l = wp_ctx.tile([128, 4 * H], BF16, tag="whh0l")
        nc.sync.dma_start(out=whh0l, in_=whh0l_d[:, :])
        whh1h = wp_ctx.tile([128, 4 * H], BF16, tag="whh1h")
        nc.sync.dma_start(out=whh1h, in_=whh1h_d[:, :])
        whh1l = wp_ctx.tile([128, 4 * H], BF16, tag="whh1l")
        nc.sync.dma_start(out=whh1l, in_=whh1l_d[:, :])
        whh = [(whh0h, whh0l), (whh1h, whh1l)]
        wfc = wp_ctx.tile([128, 2 * O], MMDT, tag="wfc")
        nc.sync.dma_start(out=wfc, in_=wfc_d[:, :])
        bias8 = wp_ctx.tile([128, 8], FP32, tag="bias8")
        nc.sync.dma_start(out=bias8, in_=bias8_d[:, :])
        bfc = wp_ctx.tile([O, 1], FP32, tag="bfc")
        nc.sync.dma_start(out=bfc, in_=bfc_d[:, :])

        # warmup x pair tiles (bf16 hi/lo)
        xpt = []
        for p in range(n_warm // 2):
            xth = xp_ctx.tile([128, 2 * BL], BF16, tag=f"xh{p}")
            nc.sync.dma_start(out=xth, in_=xrh_d[p, :, :])
            xtl = xp_ctx.tile([128, 2 * BL], BF16, tag=f"xl{p}")
            nc.sync.dma_start(out=xtl, in_=xrl_d[p, :, :])
            xpt.append((xth, xtl))
        # LA x tiles (fp32; rows 0:6 of each band get the fed-back output)
        xlat = []
        for k in range(n_la):
            xtile = xp_ctx.tile([128, BL], FP32, tag=f"xla{k}")
            nc.sync.dma_start(out=xtile, in_=xla_d[k, :, :])
            xlat.append(xtile)

        c_prev = None  # step 0 skips the f gate entirely
        h_prev = None  # step 0 skips the W_hh matmuls entirely

        # gate order i,f,g,o over M-tiles m=0..7 (gate X -> tiles 2X, 2X+1)
        # PSUM: one bank per M-tile (separate accumulation state per tile)
        GATE_FUNC = [AF.Sigmoid, AF.Sigmoid, AF.Tanh, AF.Sigmoid]  # i, f, g, o
        EMIT_ORDER = [2, 1, 0, 3]  # g, f, i, o

        def alloc_banks():
            banks = []
            for b in range(8):
                pbank = gp_ctx.tile([128, FD], FP32, tag=f"pb{b}")
                banks.append(pbank)
            return banks

        def gates_pair(xpair):
            # x-projection for TWO steps at once (N=512), quadrant-packed,
            # as three exact bf16 hi/lo products
            xth, xtl = xpair
            banks = alloc_banks()
            prods = [(wrh, xth), (wrl, xth), (wrh, xtl)]
            for j, (w_t, x_t) in enumerate(prods):
                for grp in (0, 1):
                    for band in range(4):
                        m = 4 * grp + band
                        nc.tensor.matmul(
                            banks[m][:, :],
                            w_t[32 * band : 32 * band + 32, 128 * m : 128 * m + 128],
                            x_t[32 * band : 32 * band + 32, :],
                            start=(j == 0),
                            stop=False,
                            tile_position=(32 * band, 0),
                            skip_group_check=True,
                        )
            return banks

        def x_mms(banks, xslice, first=False, x_last=False):
            # x_last: the x matmuls close accumulation groups the h matmuls
            # opened (LA phase: the o-dependent x wait hides under the h part)
            for grp in (0, 1):
                for band in range(4):
                    m = 4 * grp + band
                    if first and m in (2, 3):
                        continue  # f gate unused at step 0
                    nc.tensor.matmul(
                        banks[m][:, 0:BL],
                        wrep[32 * band : 32 * band + 32, 128 * m : 128 * m + 128],
                        xslice[32 * band : 32 * band + 32, :],
                        start=not x_last,
                        stop=first or x_last,
                        tile_position=(32 * band, 0),
                        skip_group_check=True,
                    )

        def gates_single(xpair, off, first=False):
            # warmup unpaired step: bf16 hi/lo products on a pair half
            xth, xtl = xpair
            banks = alloc_banks()
            prods = [(wrh, xth), (wrl, xth), (wrh, xtl)]
            for j, (w_t, x_t) in enumerate(prods):
                for grp in (0, 1):
                    for band in range(4):
                        m = 4 * grp + band
                        if first and m in (2, 3):
                            continue  # f gate unused at step 0
                        nc.tensor.matmul(
                            banks[m][:, 0:BL],
                            w_t[32 * band : 32 * band + 32, 128 * m : 128 * m + 128],
                            x_t[32 * band : 32 * band + 32, off : off + BL],
                            start=(j == 0),
                            stop=(first and j == 2),
                            tile_position=(32 * band, 0),
                            skip_group_check=True,
                        )
            return banks

        def gates_la(xtile):
            banks = alloc_banks()
            x_mms(banks, xtile)
            return banks

        def lstm_tail(banks, off, h_prev, c_prev, first=False, h_starts=False,
                      x_emit=None, want_bits=False):
            def psl(m):
                return banks[m][:, off : off + BL]

            # h part: per emission-ordered gate, 2 M-tiles x 2 K-tiles.
            # h lives in two half tiles so k0 matmuls start as soon as the
            # low half of the tail finishes. With h_starts=True the k0 matmul
            # opens the accumulation group (LA: x part accumulates last).
            if not first:
                for g in EMIT_ORDER:
                    for m in (2 * g, 2 * g + 1):
                        col = 128 * m
                        for k in (0, 1):
                            wh, wl = whh[k]
                            hhi, hlo = h_prev[k]
                            prods = [(wh, hhi), (wl, hhi), (wh, hlo)]
                            for j, (w_t, h_t) in enumerate(prods):
                                last = k == 1 and j == 2
                                nc.tensor.matmul(
                                    psl(m),
                                    w_t[:, col : col + 128],
                                    h_t[:, :],
                                    start=(h_starts and k == 0 and j == 0),
                                    stop=(last and not h_starts),
                                    skip_group_check=True,
                                )
            if x_emit is not None:
                x_emit()

            # activations: one instr per M-tile into per-half tiles, bias via
            # the ACT bias port
            act = {}
            for g in EMIT_ORDER:
                if first and g == 1:
                    continue
                for half in (0, 1):
                    m = 2 * g + half
                    ah = sp_ctx.tile([128, BL], FP32, tag=f"a{g}_{half}")
                    act[(g, half)] = ah
                    nc.scalar.activation(
                        out=ah,
                        in_=psl(m),
                        func=GATE_FUNC[g],
                        bias=bias8[:, m : m + 1],
                    )

            # elementwise tail, low half first so h_lo lands early
            c_new = []
            h_new = []
            bits_new = []
            for half in (0, 1):
                cn = sp_ctx.tile([128, BL], FP32, tag=f"c{half}")
                if first:
                    nc.vector.tensor_tensor(
                        out=cn, in0=act[(0, half)], in1=act[(2, half)], op=ALU.mult
                    )
                else:
                    t1 = sp_ctx.tile([128, BL], FP32, tag=f"t1_{half}")
                    nc.vector.tensor_tensor(
                        out=t1, in0=act[(1, half)], in1=c_prev[half], op=ALU.mult
                    )
                    t2 = sp_ctx.tile([128, BL], FP32, tag=f"t2_{half}")
                    nc.vector.tensor_tensor(
                        out=t2, in0=act[(0, half)], in1=act[(2, half)], op=ALU.mult
                    )
                    nc.vector.tensor_tensor(out=cn, in0=t1, in1=t2, op=ALU.add)
                c_new.append(cn)
                if want_bits:
                    bt = sp_ctx.tile([128, BL], MMDT, tag=f"bits{half}")
                    nc.vector.tensor_scalar(
                        out=bt, in0=cn, scalar1=0.0, scalar2=None, op0=ALU.is_gt
                    )
                    bits_new.append(bt)
                tc_h = sp_ctx.tile([128, BL], FP32, tag=f"tc{half}")
                nc.scalar.activation(out=tc_h, in_=cn, func=AF.Tanh)
                hhi = sp_ctx.tile([128, BL], BF16, tag=f"hhi{half}")
                nc.vector.tensor_tensor(
                    out=hhi, in0=act[(3, half)], in1=tc_h, op=ALU.mult
                )
                hn = sp_ctx.tile([128, BL], FP32, tag=f"h{half}")
                nc.vector.tensor_tensor(out=hn, in0=act[(3, half)], in1=tc_h, op=ALU.mult)
                hlo = sp_ctx.tile([128, BL], BF16, tag=f"hlo{half}")
                nc.vector.scalar_tensor_tensor(
                    out=hlo,
                    in0=hhi,
                    scalar=-1.0,
                    in1=hn,
                    op0=ALU.mult,
                    op1=ALU.add,
                )
                h_new.append((hhi, hlo))
            return h_new, c_new, bits_new

        def emit_output(k, bits_cur):
            # bits = (c' > 0); equals STE(h) since sigmoid(o)>0, tanh sign-pres.
            # po reuses a gate bank slot: all gate reads of this step are
            # done before bits is ready, so the WAR dep is satisfied
            po = gp_ctx.tile([O, BL], FP32, tag="pb0")
            for half in (0, 1):
                nc.tensor.matmul(
                    po,
                    wfc[:, O * half : O * half + O],
                    bits_cur[half][:, :],
                    start=(half == 0),
                    stop=(half == 1),
                    skip_group_check=True,
                )
            osb = sp_ctx.tile([O, BL], FP32, tag="osb")
            nc.scalar.activation(out=osb, in_=po, func=AF.Identity, bias=bfc)
            nc.sync.dma_start(out=out_d[k, :, :], in_=osb)
            return osb

        # steps 0 and 1 unpaired (step 0 has no h part), pairs from step 2
        bk = gates_single(xpt[0], 0, first=True)
        h_prev, c_prev, _ = lstm_tail(bk, 0, None, None, first=True)
        bk = gates_single(xpt[0], BL)
        h_prev, c_prev, _ = lstm_tail(bk, 0, h_prev, c_prev)
        for p in range(1, n_warm // 2):
            bk = gates_pair(xpt[p])
            h_prev, c_prev, _ = lstm_tail(bk, 0, h_prev, c_prev)
            h_prev, c_prev, bits = lstm_tail(
                bk, BL, h_prev, c_prev, want_bits=(p == n_warm // 2 - 1)
            )

        for k in range(n_la + 1):
            osb = emit_output(k, bits)
            if k < n_la:
                xv = xlat[k]
                for band in range(4):
                    dst = xv[32 * band : 32 * band + O, :]
                    if band % 2 == 0:
                        nc.vector.tensor_copy(out=dst, in_=osb)
                    else:
                        nc.scalar.copy(out=dst, in_=osb)
                bk = gates_la(xv)
                h_prev, c_prev, bits = lstm_tail(
                    bk, 0, h_prev, c_prev, want_bits=True
                )

    if spill:
        _spill_excess_waits(nc)
    return nc


def _host_prep(x, W_ih, W_hh, b_ih, b_hh, W_fc, b_fc):
    """Build the 8 per-core input maps."""
    x = np.asarray(x, dtype=np.float32)
    W_ih = np.asarray(W_ih, dtype=np.float32)
    W_hh = np.asarray(W_hh, dtype=np.float32)
    b_ih = np.asarray(b_ih, dtype=np.float32)
    b_hh = np.asarray(b_hh, dtype=np.float32)
    W_fc = np.asarray(W_fc, dtype=np.float32)
    b_fc = np.asarray(b_fc, dtype=np.float32)
    import ml_dtypes as mld

    bias = (b_ih + b_hh).astype(np.float32)
    w32t = np.ascontiguousarray(W_ih.T).astype(np.float32)  # [32, 1024]
    wrep = np.ascontiguousarray(np.tile(w32t, (4, 1)))  # [128, 1024]
    w_hi = w32t.astype(mld.bfloat16)
    w_lo = (w32t - w_hi.astype(np.float32)).astype(mld.bfloat16)
    import ml_dtypes

    whh_t = np.ascontiguousarray(W_hh.T).astype(np.float32)  # [256, 1024]
    whh_hi = whh_t.astype(ml_dtypes.bfloat16)
    whh_lo = (whh_t - whh_hi.astype(np.float32)).astype(ml_dtypes.bfloat16)
    wfc_fold = np.concatenate([W_fc.T[:128], W_fc.T[128:]], axis=1)  # [128, 12]
    shared = {
        "wrep": wrep,
        "wrh": np.ascontiguousarray(np.tile(w_hi, (4, 1))),
        "wrl": np.ascontiguousarray(np.tile(w_lo, (4, 1))),
        "whh0h": np.ascontiguousarray(whh_hi[:128]),
        "whh0l": np.ascontiguousarray(whh_lo[:128]),
        "whh1h": np.ascontiguousarray(whh_hi[128:]),
        "whh1l": np.ascontiguousarray(whh_lo[128:]),
        "wfc": np.ascontiguousarray(wfc_fold).astype(MMNP),
        "bias8": np.ascontiguousarray(bias.reshape(8, 128).T).astype(np.float32),
        "bfc": np.ascontiguousarray(b_fc.reshape(O, 1)).astype(np.float32),
    }
    in_maps = []
    for c in range(NCORES):
        xc = x[c * BL : (c + 1) * BL]  # [BL, S, F]
        xT = np.ascontiguousarray(xc.transpose(1, 2, 0)).astype(np.float32)
        xpair = (
            xT.reshape(S // 2, 2, F, BL).transpose(0, 2, 1, 3).reshape(S // 2, F, 2 * BL)
        )
        x_hi = xpair.astype(mld.bfloat16)
        x_lo = (xpair - x_hi.astype(np.float32)).astype(mld.bfloat16)
        xla = np.ascontiguousarray(np.tile(xT[:LA], (1, 4, 1)))  # [LA, 128, BL]
        in_maps.append(
            {
                "xrh": np.ascontiguousarray(np.tile(x_hi, (1, 4, 1))),
                "xrl": np.ascontiguousarray(np.tile(x_lo, (1, 4, 1))),
                "xla": xla,
                **shared,
            }
        )
    return in_maps


_NC_CACHE = {}


def _get_nc():
    if "nc" not in _NC_CACHE:
        _NC_CACHE["nc"] = build_nc()
    return _NC_CACHE["nc"]


def run(inputs, trace=False):
    in_maps = _host_prep(**inputs)
    nc = _get_nc()
    res = run_bass_kernel_spmd(nc, in_maps, core_ids=list(range(NCORES)), trace=trace)
    outs = []
    for c in range(NCORES):
        o = res.results[c]["out_t"]  # [33, 6, BL]
        outs.append(np.ascontiguousarray(o.transpose(2, 0, 1)))  # [BL, 33, 6]
    full = np.concatenate(outs, axis=0).astype(np.float32)  # [B, 33, 6]
    return full, res


def kernel(**inputs):
    full, _ = run(inputs, trace=False)
    return full


if __name__ == "__main__":
    t = build_nc()
    print("built ok")
